# revision 2
# baseline (speedup 1.0000x reference)
"""Trainium2 Bass kernel for attention-weighted pooling.

Computes, for x[B,T,D], W[D,1], b[T,1]:
    et = tanh(x @ W + b)            # (B, T)
    at = softmax(et, axis=-1)       # (B, T)
    out = einsum('btd,bt->bd', x, at)

Pure data parallel over batch across 8 NeuronCores (4 batches/core);
W and b replicated; no collectives.

Structure (per core, single streaming pass over x -> memory roofline):
  - tanh output is bounded in [-1,1] so softmax needs no max subtraction;
    normalization is deferred to a per-batch epilogue, so x is read from
    HBM exactly once (32 MiB fp32/core) and cast fp32->bf16 in the DMA
    (SWDGE) for 1-pass PE matmuls and 2x-rate DVE elementwise.
  - Per chunk of nj*128 t-rows ("(j p) d" layout, 2 KiB descriptors):
    elin[t] = sum_d x[t,d]*W[d] is split between DVE (scalar_tensor_tensor
    with accum, ~0.6us) and ACT (DVE mul + activation-Copy accum, ~0.8us)
    so neither engine exceeds the ~5.4us/chunk DMA cadence; alternating
    4/3 split by chunk parity. ACT then does tanh and exp (p in bf16),
    and PE accumulates p.T @ x_tile into a PSUM [1, D] accumulator.
  - b handling: b column c is the contiguous run b[c*128:(c+1)*128],
    loaded in ONE 512B-descriptor DMA issued before x floods the SDMA
    rings (32B-descriptor per-chunk b loads measured ~40us arrival!).
    When b is all zeros (this problem's setup_inputs), a program variant
    without the +b add is used: the add's cross-engine wait on ACT's last
    accumulator read cost ~0.7us/chunk.
  - Per-batch epilogue (S = sum_t p via ones-matmul, out = acc/S) is
    emitted one chunk into the NEXT batch so the in-order DVE queue never
    stalls on ACT's exp; the final scale is split DVE/ACT half-row each.
  - The last batch's chunk plan tapers (8,8,8,4,2,1,1 subtiles) so the
    serial DVE->ACT->PE chain after the final DMA is short.
"""

import sys

sys.path.insert(0, "/opt/trn_rl_repo")

import numpy as np

B, T, D = 32, 4096, 512
N_CORES = 8
B_LOCAL = B // N_CORES
P = 128

# chunk plans, in subtiles of 128 t-rows (sum per batch must be T//P = 32)
FULL_PLAN = [8, 8, 8, 8]
TAPER_PLAN = [8, 8, 8, 4, 2, 1, 1]

# dot-product strategy: number of leading subtiles per chunk on the
# DVE mul + ACT accum path; the rest use DVE scalar_tensor_tensor.
# 3 keeps the ACT accum chain (~3.4us) shorter than DVE's mul+stt chain
# (~4.0us), so the +b add never cross-engine-waits on ACT.
N_ACT = 4

_PROGRAM = {}


def _build_program(with_b=True):
    import concourse.bacc as bacc
    import concourse.mybir as mybir
    import concourse.tile as tile

    f32 = mybir.dt.float32
    bf16 = mybir.dt.float16
    nc = bacc.Bacc("TRN2", target_bir_lowering=False, debug=False)

    x_d = nc.dram_tensor("x", [B_LOCAL, T, D], f32, kind="ExternalInput")
    W_d = nc.dram_tensor("W", [D, 1], f32, kind="ExternalInput")
    b_d = nc.dram_tensor("b", [T, 1], f32, kind="ExternalInput")
    o_d = nc.dram_tensor("out", [B_LOCAL, D], f32, kind="ExternalOutput")

    plans = [FULL_PLAN] * (B_LOCAL - 1) + [TAPER_PLAN]
    n_cols = T // P  # p_buf columns per batch

    with tile.TileContext(nc) as tc:
        with (
            tc.tile_pool(name="consts", bufs=1) as consts,
            tc.tile_pool(name="xin", bufs=10) as xin,
            tc.tile_pool(name="prod", bufs=6) as prod_pool,
            tc.tile_pool(name="ttrs", bufs=4) as ttr_pool,
            tc.tile_pool(name="small", bufs=4) as small,
            tc.tile_pool(name="pbuf", bufs=2) as pbuf_pool,
            tc.tile_pool(name="acc_psum", bufs=2, space="PSUM") as acc_psum_pool,
            tc.tile_pool(name="s_psum", bufs=2, space="PSUM") as s_psum_pool,
        ):
            # b first (one DMA, 512B descriptors): in the (j p) t-layout,
            # column c of b_buf is the contiguous run b[c*128 : (c+1)*128],
            # which works for every chunk plan: chunk at subtile base cb,
            # local subtile j reads column cb + j.
            b_buf = consts.tile([P, n_cols], f32)
            nc.sync.dma_start(
                b_buf[:],
                b_d.ap().rearrange("(c p) one -> p (c one)", p=P),
            )

            def chunk_dma(bb, t0, nj):
                xt = xin.tile([P, 8, D], bf16, tag="xt", name="xt")
                nc.gpsimd.dma_start(
                    xt[:, 0:nj, :],
                    x_d.ap()[bb, t0 : t0 + nj * P, :].rearrange(
                        "(j p) d -> p j d", p=P
                    ),
                )
                return xt

            first_xt = chunk_dma(0, 0, plans[0][0])

            w_bcast = consts.tile([P, D], bf16)
            nc.gpsimd.dma_start(
                w_bcast[:],
                W_d.ap().rearrange("d one -> one d").broadcast_to([P, D]),
            )
            ones_col = consts.tile([P, 1], f32)
            nc.vector.memset(ones_col[:], 1.0)

            pending_epilogue = None

            for bb in range(B_LOCAL):
                plan = plans[bb]
                bbuf = b_buf
                p_buf = pbuf_pool.tile([P, n_cols], bf16)
                acc = acc_psum_pool.tile([1, D], f32, name="acc")

                total_mm = sum(plan)
                mm_idx = 0
                cb = 0
                for ci, nj in enumerate(plan):
                    t0 = cb * P
                    if bb == 0 and ci == 0:
                        xt = first_xt
                    else:
                        xt = chunk_dma(bb, t0, nj)

                    if pending_epilogue is not None and ci == 1:
                        pending_epilogue()
                        pending_epilogue = None

                    elin = small.tile([P, 8], f32, name="elin", tag="elin")
                    n_act = min(N_ACT - (ci & 1), max(nj - 1, 0))
                    for j in range(n_act):
                        prod = prod_pool.tile([P, D], bf16, name="prod")
                        nc.vector.tensor_mul(prod[:], xt[:, j, :], w_bcast[:])
                        nc.scalar.activation(
                            prod[:],
                            prod[:],
                            mybir.ActivationFunctionType.Copy,
                            accum_out=elin[:, j : j + 1],
                        )
                    for j in range(n_act, nj):
                        scratch = ttr_pool.tile([P, D], bf16, name="scratch")
                        nc.vector.scalar_tensor_tensor(
                            out=scratch[:],
                            in0=xt[:, j, :],
                            scalar=1.0,
                            in1=w_bcast[:],
                            op0=mybir.AluOpType.mult,
                            op1=mybir.AluOpType.mult,
                            accum_out=elin[:, j : j + 1],
                        )
                    if with_b:
                        ee = small.tile([P, 8], f32, name="ee", tag="ee")
                        nc.vector.tensor_add(
                            ee[:, 0:nj], elin[:, 0:nj], bbuf[:, cb : cb + nj]
                        )
                    else:
                        ee = elin
                    et = small.tile([P, 8], f32, name="et", tag="et")
                    nc.scalar.activation(
                        et[:, 0:nj], ee[:, 0:nj], mybir.ActivationFunctionType.Tanh
                    )
                    nc.scalar.activation(
                        p_buf[:, cb : cb + nj],
                        et[:, 0:nj],
                        mybir.ActivationFunctionType.Exp,
                    )
                    for j in range(nj):
                        nc.tensor.matmul(
                            acc[:],
                            p_buf[:, cb + j : cb + j + 1],
                            xt[:, j, :],
                            start=(mm_idx == 0),
                            stop=(mm_idx == total_mm - 1),
                        )
                        mm_idx += 1
                    cb += nj

                def make_epilogue(bb=bb, p_buf=p_buf, acc=acc):
                    def epi():
                        ssum = small.tile([P, 1], f32, name="ssum", tag="ssum")
                        nc.vector.reduce_sum(
                            ssum[:], p_buf[:], axis=mybir.AxisListType.X
                        )
                        s_ps = s_psum_pool.tile([1, 1], f32, name="s_ps")
                        nc.tensor.matmul(s_ps[:], ssum[:], ones_col[:])
                        sinv = small.tile([1, 1], f32, name="sinv", tag="sinv")
                        nc.vector.reciprocal(sinv[:], s_ps[:])
                        out_sb = small.tile([1, D], f32, name="out_sb", tag="osb")
                        half = D // 2
                        nc.vector.tensor_scalar_mul(
                            out_sb[:, 0:half], acc[:, 0:half], sinv[:]
                        )
                        nc.scalar.mul(out_sb[:, half:D], acc[:, half:D], sinv[:])
                        nc.sync.dma_start(o_d.ap()[bb : bb + 1, :], out_sb[:])

                    return epi

                if bb == B_LOCAL - 1:
                    make_epilogue()()
                else:
                    pending_epilogue = make_epilogue()

    nc.compile()
    return nc


def _get_program(with_b):
    if with_b not in _PROGRAM:
        _PROGRAM[with_b] = _build_program(with_b)
    return _PROGRAM[with_b]


def _shard_inputs(x, W, b):
    x = np.ascontiguousarray(np.asarray(x, dtype=np.float32))
    W = np.ascontiguousarray(np.asarray(W, dtype=np.float32))
    b = np.ascontiguousarray(np.asarray(b, dtype=np.float32))
    return [
        {"x": x[c * B_LOCAL : (c + 1) * B_LOCAL], "W": W, "b": b}
        for c in range(N_CORES)
    ]


def _install_ntff_hook_shim():
    """The agent image's ``antenv`` lacks ``axon_hooks``, so the boot-time
    NTFF hook registration silently degrades. Recreate the module in
    sys.modules and register the ctypes hook against libaxon_pjrt.so."""
    import types

    if "antenv.axon_hooks" in sys.modules:
        return
    mod = types.ModuleType("antenv.axon_hooks")
    _hook = [None]
    mod.set_axon_ntff_profile_hook = lambda h: _hook.__setitem__(0, h)
    mod.get_axon_ntff_profile_hook = lambda: _hook[0]
    import antenv

    antenv.axon_hooks = mod
    sys.modules["antenv.axon_hooks"] = mod
    try:
        sys.path.insert(0, "/root/.axon_site")
        from trn_agent_boot.trn_boot import _ntff_profile_via_ctypes

        mod.set_axon_ntff_profile_hook(
            _ntff_profile_via_ctypes("/opt/axon/libaxon_pjrt.so")
        )
    except Exception as e:  # profiling is best-effort; run still works
        print(f"NTFF hook shim failed ({e}); tracing disabled", file=sys.stderr)


def _run(in_maps, trace=False, with_b=True):
    from concourse.bass_utils import run_bass_kernel_spmd

    nc = _get_program(with_b)
    kwargs = {}
    if trace:
        _install_ntff_hook_shim()
        kwargs = {"trace": True, "trace_cores": [0]}
    return run_bass_kernel_spmd(nc, in_maps, core_ids=list(range(N_CORES)), **kwargs)


def kernel(x, W, b):
    with_b = bool(np.any(np.asarray(b)))
    res = _run(_shard_inputs(x, W, b), with_b=with_b)
    return np.concatenate(
        [res.results[c]["out"] for c in range(N_CORES)], axis=0
    ).astype(np.float32)


def kernel_profiled(x, W, b):
    """Like kernel() but also returns the NTFF-measured exec time in ns."""
    with_b = bool(np.any(np.asarray(b)))
    res = _run(_shard_inputs(x, W, b), trace=True, with_b=with_b)
    out = np.concatenate(
        [res.results[c]["out"] for c in range(N_CORES)], axis=0
    ).astype(np.float32)
    return out, res


# revision 3
# speedup vs baseline: 1.2077x; 1.2077x over previous
"""Trainium2 Bass kernel for attention-weighted pooling.

Computes, for x[B,T,D], W[D,1], b[T,1]:
    et = tanh(x @ W + b)            # (B, T)
    at = softmax(et, axis=-1)       # (B, T)
    out = einsum('btd,bt->bd', x, at)

Pure data parallel over batch across 8 NeuronCores (4 batches/core);
W and b replicated; no collectives.

Structure (per core, single streaming pass over x -> memory roofline):
  - tanh output is bounded in [-1,1] so softmax needs no max subtraction;
    normalization is deferred to a per-batch epilogue, so x is read from
    HBM exactly once (32 MiB fp32/core) and cast fp32->bf16 in the DMA
    (SWDGE) for 1-pass PE matmuls and 2x-rate DVE elementwise.
  - Per chunk of nj*128 t-rows ("(j p) d" layout, 2 KiB descriptors):
    elin[t] = sum_d x[t,d]*W[d] is split between DVE (scalar_tensor_tensor
    with accum, ~0.6us) and ACT (DVE mul + activation-Copy accum, ~0.8us)
    so neither engine exceeds the ~5.4us/chunk DMA cadence; 4 subtiles on
    each path balances DVE (~3.9us) vs ACT (~3.6us) per 8-subtile chunk.
    ACT then does tanh and exp (p in bf16),
    and PE accumulates p.T @ x_tile into a PSUM [1, D] accumulator.
  - b handling: b column c is the contiguous run b[c*128:(c+1)*128],
    loaded in ONE 512B-descriptor DMA issued before x floods the SDMA
    rings (32B-descriptor per-chunk b loads measured ~40us arrival!).
    When b is all zeros (this problem's setup_inputs), a program variant
    without the +b add is used: the add's cross-engine wait on ACT's last
    accumulator read cost ~0.7us/chunk.
  - Per-batch epilogue (S = sum_t p via ones-matmul, out = acc/S) is
    emitted one chunk into the NEXT batch so the in-order DVE queue never
    stalls on ACT's exp; the final scale is split DVE/ACT half-row each.
  - The last batch's chunk plan tapers (8,8,8,4,2,1,1 subtiles) so the
    serial DVE->ACT->PE chain after the final DMA is short.
"""

import sys

sys.path.insert(0, "/opt/trn_rl_repo")

import numpy as np

B, T, D = 32, 4096, 512
N_CORES = 8
B_LOCAL = B // N_CORES
P = 128

# chunk plans, in subtiles of 128 t-rows (sum per batch must be T//P = 32)
FULL_PLAN = [8, 8, 8, 8]
TAPER_PLAN = [8, 8, 8, 4, 2, 1, 1]

# subtiles per chunk on the DVE-mul + ACT-accum path; the rest use DVE
# scalar_tensor_tensor. 4/4 equalizes DVE (~3.9us) and ACT (~3.6us) per
# chunk, keeping both under the DMA cadence at any DVFS state.
N_ACT = 4

_PROGRAM = {}


def _build_program(with_b=True):
    import concourse.bacc as bacc
    import concourse.mybir as mybir
    import concourse.tile as tile

    f32 = mybir.dt.float32
    bf16 = mybir.dt.float16
    nc = bacc.Bacc("TRN2", target_bir_lowering=False, debug=False)

    x_d = nc.dram_tensor("x", [B_LOCAL, T, D], f32, kind="ExternalInput")
    W_d = nc.dram_tensor("W", [D, 1], f32, kind="ExternalInput")
    b_d = nc.dram_tensor("b", [T, 1], f32, kind="ExternalInput")
    o_d = nc.dram_tensor("out", [B_LOCAL, D], f32, kind="ExternalOutput")

    plans = [FULL_PLAN] * (B_LOCAL - 1) + [TAPER_PLAN]
    n_cols = T // P  # p_buf columns per batch

    with tile.TileContext(nc) as tc:
        with (
            tc.tile_pool(name="consts", bufs=1) as consts,
            tc.tile_pool(name="xin", bufs=10) as xin,
            tc.tile_pool(name="prod", bufs=6) as prod_pool,
            tc.tile_pool(name="ttrs", bufs=4) as ttr_pool,
            tc.tile_pool(name="small", bufs=4) as small,
            tc.tile_pool(name="pbuf", bufs=2) as pbuf_pool,
            tc.tile_pool(name="acc_psum", bufs=2, space="PSUM") as acc_psum_pool,
            tc.tile_pool(name="s_psum", bufs=2, space="PSUM") as s_psum_pool,
        ):
            # b first (one DMA, 512B descriptors): in the (j p) t-layout,
            # column c of b_buf is the contiguous run b[c*128 : (c+1)*128],
            # which works for every chunk plan: chunk at subtile base cb,
            # local subtile j reads column cb + j.
            b_buf = consts.tile([P, n_cols], f32)
            nc.sync.dma_start(
                b_buf[:],
                b_d.ap().rearrange("(c p) one -> p (c one)", p=P),
            )

            def chunk_dma(bb, t0, nj):
                xt = xin.tile([P, 8, D], bf16, tag="xt", name="xt")
                nc.gpsimd.dma_start(
                    xt[:, 0:nj, :],
                    x_d.ap()[bb, t0 : t0 + nj * P, :].rearrange(
                        "(j p) d -> p j d", p=P
                    ),
                )
                return xt

            first_xt = chunk_dma(0, 0, plans[0][0])

            w_bcast = consts.tile([P, D], bf16)
            nc.gpsimd.dma_start(
                w_bcast[:],
                W_d.ap().rearrange("d one -> one d").broadcast_to([P, D]),
            )
            ones_col = consts.tile([P, 1], f32)
            nc.vector.memset(ones_col[:], 1.0)

            pending_epilogue = None

            for bb in range(B_LOCAL):
                plan = plans[bb]
                bbuf = b_buf
                p_buf = pbuf_pool.tile([P, n_cols], bf16)
                acc = acc_psum_pool.tile([1, D], f32, name="acc")

                total_mm = sum(plan)
                mm_idx = 0
                cb = 0
                for ci, nj in enumerate(plan):
                    t0 = cb * P
                    if bb == 0 and ci == 0:
                        xt = first_xt
                    else:
                        xt = chunk_dma(bb, t0, nj)

                    if pending_epilogue is not None and ci == 1:
                        pending_epilogue()
                        pending_epilogue = None

                    elin = small.tile([P, 8], f32, name="elin", tag="elin")
                    n_act = min(N_ACT, max(nj - 1, 0))
                    for j in range(n_act):
                        prod = prod_pool.tile([P, D], bf16, name="prod")
                        nc.vector.tensor_mul(prod[:], xt[:, j, :], w_bcast[:])
                        nc.scalar.activation(
                            prod[:],
                            prod[:],
                            mybir.ActivationFunctionType.Copy,
                            accum_out=elin[:, j : j + 1],
                        )
                    for j in range(n_act, nj):
                        scratch = ttr_pool.tile([P, D], bf16, name="scratch")
                        nc.vector.scalar_tensor_tensor(
                            out=scratch[:],
                            in0=xt[:, j, :],
                            scalar=1.0,
                            in1=w_bcast[:],
                            op0=mybir.AluOpType.mult,
                            op1=mybir.AluOpType.mult,
                            accum_out=elin[:, j : j + 1],
                        )
                    if with_b:
                        ee = small.tile([P, 8], f32, name="ee", tag="ee")
                        nc.vector.tensor_add(
                            ee[:, 0:nj], elin[:, 0:nj], bbuf[:, cb : cb + nj]
                        )
                    else:
                        ee = elin
                    et = small.tile([P, 8], f32, name="et", tag="et")
                    nc.scalar.activation(
                        et[:, 0:nj], ee[:, 0:nj], mybir.ActivationFunctionType.Tanh
                    )
                    nc.scalar.activation(
                        p_buf[:, cb : cb + nj],
                        et[:, 0:nj],
                        mybir.ActivationFunctionType.Exp,
                    )
                    for j in range(nj):
                        nc.tensor.matmul(
                            acc[:],
                            p_buf[:, cb + j : cb + j + 1],
                            xt[:, j, :],
                            start=(mm_idx == 0),
                            stop=(mm_idx == total_mm - 1),
                        )
                        mm_idx += 1
                    cb += nj

                def make_epilogue(bb=bb, p_buf=p_buf, acc=acc):
                    def epi():
                        ssum = small.tile([P, 1], f32, name="ssum", tag="ssum")
                        nc.vector.reduce_sum(
                            ssum[:], p_buf[:], axis=mybir.AxisListType.X
                        )
                        s_ps = s_psum_pool.tile([1, 1], f32, name="s_ps")
                        nc.tensor.matmul(s_ps[:], ssum[:], ones_col[:])
                        sinv = small.tile([1, 1], f32, name="sinv", tag="sinv")
                        nc.vector.reciprocal(sinv[:], s_ps[:])
                        out_sb = small.tile([1, D], f32, name="out_sb", tag="osb")
                        half = D // 2
                        nc.vector.tensor_scalar_mul(
                            out_sb[:, 0:half], acc[:, 0:half], sinv[:]
                        )
                        nc.scalar.mul(out_sb[:, half:D], acc[:, half:D], sinv[:])
                        nc.sync.dma_start(o_d.ap()[bb : bb + 1, :], out_sb[:])

                    return epi

                if bb == B_LOCAL - 1:
                    make_epilogue()()
                else:
                    pending_epilogue = make_epilogue()

    nc.compile()
    return nc


def _get_program(with_b):
    if with_b not in _PROGRAM:
        _PROGRAM[with_b] = _build_program(with_b)
    return _PROGRAM[with_b]


def _shard_inputs(x, W, b):
    x = np.ascontiguousarray(np.asarray(x, dtype=np.float32))
    W = np.ascontiguousarray(np.asarray(W, dtype=np.float32))
    b = np.ascontiguousarray(np.asarray(b, dtype=np.float32))
    return [
        {"x": x[c * B_LOCAL : (c + 1) * B_LOCAL], "W": W, "b": b}
        for c in range(N_CORES)
    ]


def _install_ntff_hook_shim():
    """The agent image's ``antenv`` lacks ``axon_hooks``, so the boot-time
    NTFF hook registration silently degrades. Recreate the module in
    sys.modules and register the ctypes hook against libaxon_pjrt.so."""
    import types

    if "antenv.axon_hooks" in sys.modules:
        return
    mod = types.ModuleType("antenv.axon_hooks")
    _hook = [None]
    mod.set_axon_ntff_profile_hook = lambda h: _hook.__setitem__(0, h)
    mod.get_axon_ntff_profile_hook = lambda: _hook[0]
    import antenv

    antenv.axon_hooks = mod
    sys.modules["antenv.axon_hooks"] = mod
    try:
        sys.path.insert(0, "/root/.axon_site")
        from trn_agent_boot.trn_boot import _ntff_profile_via_ctypes

        mod.set_axon_ntff_profile_hook(
            _ntff_profile_via_ctypes("/opt/axon/libaxon_pjrt.so")
        )
    except Exception as e:  # profiling is best-effort; run still works
        print(f"NTFF hook shim failed ({e}); tracing disabled", file=sys.stderr)


def _run(in_maps, trace=False, with_b=True):
    from concourse.bass_utils import run_bass_kernel_spmd

    nc = _get_program(with_b)
    kwargs = {}
    if trace:
        _install_ntff_hook_shim()
        kwargs = {"trace": True, "trace_cores": [0]}
    return run_bass_kernel_spmd(nc, in_maps, core_ids=list(range(N_CORES)), **kwargs)


def kernel(x, W, b):
    with_b = bool(np.any(np.asarray(b)))
    res = _run(_shard_inputs(x, W, b), with_b=with_b)
    return np.concatenate(
        [res.results[c]["out"] for c in range(N_CORES)], axis=0
    ).astype(np.float32)


def kernel_profiled(x, W, b):
    """Like kernel() but also returns the NTFF-measured exec time in ns."""
    with_b = bool(np.any(np.asarray(b)))
    res = _run(_shard_inputs(x, W, b), trace=True, with_b=with_b)
    out = np.concatenate(
        [res.results[c]["out"] for c in range(N_CORES)], axis=0
    ).astype(np.float32)
    return out, res


# revision 4
# speedup vs baseline: 1.2261x; 1.0152x over previous
"""Trainium2 Bass kernel for attention-weighted pooling.

Computes, for x[B,T,D], W[D,1], b[T,1]:
    et = tanh(x @ W + b)            # (B, T)
    at = softmax(et, axis=-1)       # (B, T)
    out = einsum('btd,bt->bd', x, at)

Pure data parallel over batch across 8 NeuronCores (4 batches/core);
W and b replicated; no collectives.

Structure (per core, single streaming pass over x -> memory roofline):
  - tanh output is bounded in [-1,1] so softmax needs no max subtraction;
    normalization is deferred to a per-batch epilogue, so x is read from
    HBM exactly once (32 MiB fp32/core) and cast fp32->bf16 in the DMA
    (SWDGE) for 1-pass PE matmuls and 2x-rate DVE elementwise.
  - Per chunk of nj*128 t-rows ("(j p) d" layout, 2 KiB descriptors):
    elin[t] = sum_d x[t,d]*W[d] is split between DVE (scalar_tensor_tensor
    with accum, ~0.6us) and ACT (DVE mul + activation-Copy accum, ~0.8us)
    so neither engine exceeds the ~5.4us/chunk DMA cadence; 4 subtiles on
    each path balances DVE (~3.9us) vs ACT (~3.6us) per 8-subtile chunk.
    ACT then does tanh and exp (p in bf16),
    and PE accumulates p.T @ x_tile into a PSUM [1, D] accumulator.
  - b handling: b column c is the contiguous run b[c*128:(c+1)*128],
    loaded in ONE 512B-descriptor DMA issued before x floods the SDMA
    rings (32B-descriptor per-chunk b loads measured ~40us arrival!).
    When b is all zeros (this problem's setup_inputs), a program variant
    without the +b add is used: the add's cross-engine wait on ACT's last
    accumulator read cost ~0.7us/chunk.
  - Per-batch epilogue (S = sum_t p via ones-matmul, out = acc/S) is
    emitted one chunk into the NEXT batch so the in-order DVE queue never
    stalls on ACT's exp; the final scale is split DVE/ACT half-row each.
  - The last batch's chunk plan tapers (8,8,8,4,2,1,1 subtiles) so the
    serial DVE->ACT->PE chain after the final DMA is short.
"""

import sys

sys.path.insert(0, "/opt/trn_rl_repo")

import numpy as np

B, T, D = 32, 4096, 512
N_CORES = 8
B_LOCAL = B // N_CORES
P = 128

# chunk plans, in subtiles of 128 t-rows (sum per batch must be T//P = 32)
FULL_PLAN = [8, 8, 8, 8]
TAPER_PLAN = [8, 8, 8, 4, 2, 1, 1]

# subtiles per chunk on the DVE-mul + ACT-accum path; the rest use DVE
# scalar_tensor_tensor. 4/4 equalizes DVE (~3.9us) and ACT (~3.6us) per
# chunk, keeping both under the DMA cadence at any DVFS state.
N_ACT = 4

_PROGRAM = {}


def _build_program(with_b=True):
    import concourse.bacc as bacc
    import concourse.mybir as mybir
    import concourse.tile as tile

    f32 = mybir.dt.float32
    bf16 = mybir.dt.float16
    nc = bacc.Bacc("TRN2", target_bir_lowering=False, debug=False)

    x_d = nc.dram_tensor("x", [B_LOCAL, T, D], f32, kind="ExternalInput")
    W_d = nc.dram_tensor("W", [D, 1], f32, kind="ExternalInput")
    b_d = nc.dram_tensor("b", [T, 1], f32, kind="ExternalInput")
    o_d = nc.dram_tensor("out", [B_LOCAL, D], f32, kind="ExternalOutput")

    plans = [FULL_PLAN] * (B_LOCAL - 1) + [TAPER_PLAN]
    n_cols = T // P  # p_buf columns per batch

    with tile.TileContext(nc) as tc:
        with (
            tc.tile_pool(name="consts", bufs=1) as consts,
            tc.tile_pool(name="xin", bufs=10) as xin,
            tc.tile_pool(name="prod", bufs=6) as prod_pool,
            tc.tile_pool(name="ttrs", bufs=4) as ttr_pool,
            tc.tile_pool(name="small", bufs=4) as small,
            tc.tile_pool(name="pbuf", bufs=2) as pbuf_pool,
            tc.tile_pool(name="acc_psum", bufs=2, space="PSUM") as acc_psum_pool,
            tc.tile_pool(name="s_psum", bufs=2, space="PSUM") as s_psum_pool,
        ):
            # b first (one DMA, 512B descriptors): in the (j p) t-layout,
            # column c of b_buf is the contiguous run b[c*128 : (c+1)*128],
            # which works for every chunk plan: chunk at subtile base cb,
            # local subtile j reads column cb + j.
            b_buf = consts.tile([P, n_cols], f32)
            nc.sync.dma_start(
                b_buf[:],
                b_d.ap().rearrange("(c p) one -> p (c one)", p=P),
            )

            def chunk_dma(bb, t0, nj):
                xt = xin.tile([P, 8, D], bf16, tag="xt", name="xt")
                if not with_b and nj % 2 == 0:
                    # 4 KiB descriptors: column j*2+r holds t = t0 + j*256
                    # + 2p + r, so each (partition, j) line covers 2
                    # consecutive t-rows. Halves SWDGE descriptor fetches
                    # (the DMA-engine-7/15 contention) and Pool descgen.
                    # Any t<->column bijection is fine when b is zero.
                    nc.gpsimd.dma_start(
                        xt[:, 0:nj, :].rearrange("q (j r) d -> q j r d", r=2),
                        x_d.ap()[bb, t0 : t0 + nj * P, :].rearrange(
                            "(j q r) d -> q j r d", q=P, r=2
                        ),
                    )
                else:
                    nc.gpsimd.dma_start(
                        xt[:, 0:nj, :],
                        x_d.ap()[bb, t0 : t0 + nj * P, :].rearrange(
                            "(j p) d -> p j d", p=P
                        ),
                    )
                return xt

            first_xt = chunk_dma(0, 0, plans[0][0])

            w_bcast = consts.tile([P, D], bf16)
            nc.gpsimd.dma_start(
                w_bcast[:],
                W_d.ap().rearrange("d one -> one d").broadcast_to([P, D]),
            )
            ones_col = consts.tile([P, 1], f32)
            nc.vector.memset(ones_col[:], 1.0)

            pending_epilogue = None

            for bb in range(B_LOCAL):
                plan = plans[bb]
                bbuf = b_buf
                p_buf = pbuf_pool.tile([P, n_cols], bf16)
                acc = acc_psum_pool.tile([1, D], f32, name="acc")

                total_mm = sum(plan)
                mm_idx = 0
                cb = 0
                for ci, nj in enumerate(plan):
                    t0 = cb * P
                    if bb == 0 and ci == 0:
                        xt = first_xt
                    else:
                        xt = chunk_dma(bb, t0, nj)

                    if pending_epilogue is not None and ci == 1:
                        pending_epilogue()
                        pending_epilogue = None

                    elin = small.tile([P, 8], f32, name="elin", tag="elin")
                    n_act = min(N_ACT, max(nj - 1, 0))
                    for j in range(n_act):
                        prod = prod_pool.tile([P, D], bf16, name="prod")
                        nc.vector.tensor_mul(prod[:], xt[:, j, :], w_bcast[:])
                        nc.scalar.activation(
                            prod[:],
                            prod[:],
                            mybir.ActivationFunctionType.Copy,
                            accum_out=elin[:, j : j + 1],
                        )
                    for j in range(n_act, nj):
                        scratch = ttr_pool.tile([P, D], bf16, name="scratch")
                        nc.vector.scalar_tensor_tensor(
                            out=scratch[:],
                            in0=xt[:, j, :],
                            scalar=1.0,
                            in1=w_bcast[:],
                            op0=mybir.AluOpType.mult,
                            op1=mybir.AluOpType.mult,
                            accum_out=elin[:, j : j + 1],
                        )
                    if with_b:
                        ee = small.tile([P, 8], f32, name="ee", tag="ee")
                        nc.vector.tensor_add(
                            ee[:, 0:nj], elin[:, 0:nj], bbuf[:, cb : cb + nj]
                        )
                    else:
                        ee = elin
                    et = small.tile([P, 8], f32, name="et", tag="et")
                    nc.scalar.activation(
                        et[:, 0:nj], ee[:, 0:nj], mybir.ActivationFunctionType.Tanh
                    )
                    nc.scalar.activation(
                        p_buf[:, cb : cb + nj],
                        et[:, 0:nj],
                        mybir.ActivationFunctionType.Exp,
                    )
                    for j in range(nj):
                        nc.tensor.matmul(
                            acc[:],
                            p_buf[:, cb + j : cb + j + 1],
                            xt[:, j, :],
                            start=(mm_idx == 0),
                            stop=(mm_idx == total_mm - 1),
                        )
                        mm_idx += 1
                    cb += nj

                def make_epilogue(bb=bb, p_buf=p_buf, acc=acc):
                    def epi():
                        ssum = small.tile([P, 1], f32, name="ssum", tag="ssum")
                        nc.vector.reduce_sum(
                            ssum[:], p_buf[:], axis=mybir.AxisListType.X
                        )
                        s_ps = s_psum_pool.tile([1, 1], f32, name="s_ps")
                        nc.tensor.matmul(s_ps[:], ssum[:], ones_col[:])
                        sinv = small.tile([1, 1], f32, name="sinv", tag="sinv")
                        nc.vector.reciprocal(sinv[:], s_ps[:])
                        out_sb = small.tile([1, D], f32, name="out_sb", tag="osb")
                        half = D // 2
                        nc.vector.tensor_scalar_mul(
                            out_sb[:, 0:half], acc[:, 0:half], sinv[:]
                        )
                        nc.scalar.mul(out_sb[:, half:D], acc[:, half:D], sinv[:])
                        nc.sync.dma_start(o_d.ap()[bb : bb + 1, :], out_sb[:])

                    return epi

                if bb == B_LOCAL - 1:
                    make_epilogue()()
                else:
                    pending_epilogue = make_epilogue()

    nc.compile()
    return nc


def _get_program(with_b):
    if with_b not in _PROGRAM:
        _PROGRAM[with_b] = _build_program(with_b)
    return _PROGRAM[with_b]


def _shard_inputs(x, W, b):
    x = np.ascontiguousarray(np.asarray(x, dtype=np.float32))
    W = np.ascontiguousarray(np.asarray(W, dtype=np.float32))
    b = np.ascontiguousarray(np.asarray(b, dtype=np.float32))
    return [
        {"x": x[c * B_LOCAL : (c + 1) * B_LOCAL], "W": W, "b": b}
        for c in range(N_CORES)
    ]


def _install_ntff_hook_shim():
    """The agent image's ``antenv`` lacks ``axon_hooks``, so the boot-time
    NTFF hook registration silently degrades. Recreate the module in
    sys.modules and register the ctypes hook against libaxon_pjrt.so."""
    import types

    if "antenv.axon_hooks" in sys.modules:
        return
    mod = types.ModuleType("antenv.axon_hooks")
    _hook = [None]
    mod.set_axon_ntff_profile_hook = lambda h: _hook.__setitem__(0, h)
    mod.get_axon_ntff_profile_hook = lambda: _hook[0]
    import antenv

    antenv.axon_hooks = mod
    sys.modules["antenv.axon_hooks"] = mod
    try:
        sys.path.insert(0, "/root/.axon_site")
        from trn_agent_boot.trn_boot import _ntff_profile_via_ctypes

        mod.set_axon_ntff_profile_hook(
            _ntff_profile_via_ctypes("/opt/axon/libaxon_pjrt.so")
        )
    except Exception as e:  # profiling is best-effort; run still works
        print(f"NTFF hook shim failed ({e}); tracing disabled", file=sys.stderr)


def _run(in_maps, trace=False, with_b=True):
    from concourse.bass_utils import run_bass_kernel_spmd

    nc = _get_program(with_b)
    kwargs = {}
    if trace:
        _install_ntff_hook_shim()
        kwargs = {"trace": True, "trace_cores": [0]}
    return run_bass_kernel_spmd(nc, in_maps, core_ids=list(range(N_CORES)), **kwargs)


def kernel(x, W, b):
    with_b = bool(np.any(np.asarray(b)))
    res = _run(_shard_inputs(x, W, b), with_b=with_b)
    return np.concatenate(
        [res.results[c]["out"] for c in range(N_CORES)], axis=0
    ).astype(np.float32)


def kernel_profiled(x, W, b):
    """Like kernel() but also returns the NTFF-measured exec time in ns."""
    with_b = bool(np.any(np.asarray(b)))
    res = _run(_shard_inputs(x, W, b), trace=True, with_b=with_b)
    out = np.concatenate(
        [res.results[c]["out"] for c in range(N_CORES)], axis=0
    ).astype(np.float32)
    return out, res
